# revision 1
# baseline (speedup 1.0000x reference)
"""Bahdanau-style additive attention kernel for Trainium2 (8 NeuronCores).

Computes, per batch b:
    q = query[b] @ W_q.T            # [F, H]
    c = context[b] @ W_c.T          # [S, H]
    E[f, s] = sum_h v[h] * tanh(q[f, h] + c[s, h])
    out[b] = softmax(E, axis=-1)    # [F, S]

Sharding: data-parallel over batch. 16 batches -> 8 cores x 2 batches.
Each core gets its own batch slice plus the full (tiny) W_q/W_c/v.
Inputs are pre-transposed on the host (queryT/contextT/W^T) so the
contraction dim lands on SBUF partitions without on-chip transposes.

Per-core dataflow (all shapes hardcoded):
  - PE projects to qT[h, f], cT[h, s] (h on partitions, 2 h-tiles).
  - DVE builds A[h, (ht, s, f)] = cT[h, s] + qT[h, f] with stride-0
    broadcast access patterns (one [128, 8192] instruction per s-block).
  - ACT applies tanh on the big tiles (fp16 output).
  - PE reduces over h against v: per s, matmul with the fp16 tanh tile
    as stationary [h=128, f=128] and v h-tile [128, 1] as moving,
    accumulating E[:, s] columns in PSUM as [f=128, s=256].
  - A tail fraction of s-values (BIAS_S) skips the DVE add and instead
    uses ACT's fused bias: tanh(qT + cT[:, s]) per (s, h-tile).
  - Softmax: DVE reduce_max(negate) -> ACT exp(E - max) with accum_out
    row-sum -> DVE reciprocal -> DVE scale -> DMA out.
"""

import sys

for _p in ("/opt/trn_rl_repo", "/opt/pypackages"):
    if _p not in sys.path:
        sys.path.append(_p)

from contextlib import ExitStack

import numpy as np

import concourse.bass as bass
import concourse.tile as tile
from concourse import mybir

B, F, S, D, H = 16, 128, 256, 256, 256
NCORES = 8
BPC = B // NCORES  # batches per core
S_BLK = 32         # legacy default block size (build_program overrides below)
S_BLOCKS = [48, 48, 48, 48, 48, 16]  # s-block sizes per batch
ASSIST_S = 48      # leading s values whose adds run on PE (fp16 identity MMs)
BIAS_S = 0         # s values per batch routed through the ACT-bias path
T_DT = mybir.dt.float16  # dtype of tanh tiles + v (stationary path)
F16 = mybir.dt.float16
F32 = mybir.dt.float32
AF = mybir.ActivationFunctionType


def build_program(reps: int = 1, s_blocks=None, assist_s=None, interleave=False) -> bass.Bass:
    """s_blocks: list of s-block sizes (sum + BIAS_S == S); assist_s: how many
    leading s values (multiple of 8) run their adds on PE instead of DVE."""
    if s_blocks is None:
        s_blocks = list(S_BLOCKS)
    if assist_s is None:
        assist_s = ASSIST_S
    assert sum(s_blocks) + BIAS_S == S and assist_s % 8 == 0
    nc = bass.Bass()
    qT_d = nc.declare_dram_parameter("queryT", [BPC, D, F], F32, isOutput=False)
    cT_d = nc.declare_dram_parameter("contextT", [BPC, D, S], F32, isOutput=False)
    wqT_d = nc.declare_dram_parameter("w_qT", [D, H], F32, isOutput=False)
    wcT_d = nc.declare_dram_parameter("w_cT", [D, H], F32, isOutput=False)
    v_d = nc.declare_dram_parameter("v", [H, 1], F32, isOutput=False)
    out_d = nc.declare_dram_parameter("out", [BPC, F, S], F32, isOutput=True)

    n_bias = BIAS_S

    with tile.TileContext(nc) as tc, ExitStack() as ctx:
        consts = ctx.enter_context(tc.tile_pool(name="consts", bufs=1))
        loads = ctx.enter_context(tc.tile_pool(name="loads", bufs=2))
        proj = ctx.enter_context(tc.tile_pool(name="proj", bufs=2))
        work = ctx.enter_context(tc.tile_pool(name="work", bufs=2))
        work3 = ctx.enter_context(tc.tile_pool(name="work3", bufs=3))
        stats = ctx.enter_context(tc.tile_pool(name="stats", bufs=4))
        outp = ctx.enter_context(tc.tile_pool(name="outp", bufs=2))
        ps_scr = ctx.enter_context(tc.tile_pool(name="ps_scr", bufs=1, space="PSUM"))
        ps_e = ctx.enter_context(tc.tile_pool(name="ps_e", bufs=2, space="PSUM"))
        ps_a = ctx.enter_context(tc.tile_pool(name="ps_a", bufs=2, space="PSUM"))

        # v as two h-tiles: columns of a [128, 2] tile (cast to T_DT)
        v32 = consts.tile([128, 2], F32)
        for ht in range(2):
            nc.sync.dma_start(out=v32[:, ht : ht + 1], in_=v_d[128 * ht : 128 * (ht + 1), :])
        v_sb = consts.tile([128, 2], T_DT)
        nc.vector.tensor_copy(v_sb, v32)

        ident16 = None
        if assist_s:
            from concourse.masks import make_identity

            ident16 = consts.tile([128, 128], F16)
            make_identity(nc, ident16)

        # W^T tiles: [d_part, d_chunk, h]
        wqT = consts.tile([128, 2, 256], F32)
        wcT = consts.tile([128, 2, 256], F32)
        for di in range(2):
            nc.sync.dma_start(out=wqT[:, di, :], in_=wqT_d[128 * di : 128 * (di + 1), :])
            nc.sync.dma_start(out=wcT[:, di, :], in_=wcT_d[128 * di : 128 * (di + 1), :])

        def setup_batch(b):
            qryT = loads.tile([128, 2, 128], F32)
            ctxT = loads.tile([128, 2, 256], F32)
            for di in range(2):
                nc.sync.dma_start(out=qryT[:, di, :], in_=qT_d[b, 128 * di : 128 * (di + 1), :])
                nc.sync.dma_start(out=ctxT[:, di, :], in_=cT_d[b, 128 * di : 128 * (di + 1), :])
            qT = proj.tile([128, 2, 128], F32)
            cT = proj.tile([128, 2, 256], F32)
            for ht in range(2):
                qp = ps_scr.tile([128, 128], F32, tag="tp")
                for di in range(2):
                    nc.tensor.matmul(qp, lhsT=wqT[:, di, 128 * ht : 128 * (ht + 1)],
                                     rhs=qryT[:, di, :], start=(di == 0), stop=(di == 1))
                nc.vector.tensor_copy(qT[:, ht, :], qp)
                cp = ps_scr.tile([128, 256], F32, tag="cp")
                for di in range(2):
                    nc.tensor.matmul(cp, lhsT=wcT[:, di, 128 * ht : 128 * (ht + 1)],
                                     rhs=ctxT[:, di, :], start=(di == 0), stop=(di == 1))
                nc.vector.tensor_copy(cT[:, ht, :], cp)
            qT16 = cT16 = None
            if assist_s:
                qT16 = proj.tile([128, 2, 128], F16)
                cT16 = proj.tile([128, 2, 256], F16)
                nc.vector.tensor_copy(qT16, qT)
                nc.vector.tensor_copy(cT16, cT)
            e_ps = ps_e.tile([128, 256], F32)
            return dict(qT=qT, cT=cT, qT16=qT16, cT16=cT16, e_ps=e_ps)

        def do_block(st, s0, bs):
            qT, cT, qT16, cT16, e_ps = st["qT"], st["cT"], st["qT16"], st["cT16"], st["e_ps"]
            if s0 + bs <= assist_s:
                t_t = work.tile([128, 2, bs, 128], T_DT, tag="t_t")
                for ht in range(2):
                    for sq in range(0, bs, 8):
                        a_ps = ps_a.tile([128, 8, 128], F32, tag="a_ps")
                        for half in range(2):
                            sl4 = slice(4 * half, 4 * half + 4)
                            nc.tensor.matmul(a_ps[:, sl4], lhsT=ident16,
                                rhs=qT16[:, ht].unsqueeze(1).broadcast_to((128, 4, 128)),
                                start=True, stop=False)
                            nc.tensor.matmul(a_ps[:, sl4], lhsT=ident16,
                                rhs=cT16[:, ht, s0 + sq + 4 * half : s0 + sq + 4 * half + 4]
                                .unsqueeze(2).broadcast_to((128, 4, 128)),
                                start=False, stop=True)
                        nc.scalar.activation(out=t_t[:, ht, sq : sq + 8], in_=a_ps, func=AF.Tanh)
            else:
                a_t = work.tile([128, 2, bs, 128], F32)
                nc.vector.tensor_add(out=a_t,
                    in0=cT[:, :, s0 : s0 + bs].unsqueeze(3).broadcast_to((128, 2, bs, 128)),
                    in1=qT.unsqueeze(2).broadcast_to((128, 2, bs, 128)))
                t_t = work.tile([128, 2, bs, 128], T_DT, tag="t_t")
                nc.scalar.activation(out=t_t, in_=a_t, func=AF.Tanh)
            for sl in range(bs):
                s = s0 + sl
                for ht in range(2):
                    nc.tensor.matmul(e_ps[:, s : s + 1], lhsT=t_t[:, ht, sl],
                                     rhs=v_sb[:, ht : ht + 1], start=(ht == 0), stop=(ht == 1))

        starts = [0]
        for bs in s_blocks:
            starts.append(starts[-1] + bs)

        if interleave:
            for rep in range(reps):
                sts = [setup_batch(b) for b in range(BPC)]
                for i, bs in enumerate(s_blocks):
                    for b in range(BPC):
                        do_block(sts[b], starts[i], bs)
                for b in range(BPC):
                    st = sts[b]
                    qT, cT, e_ps = st["qT"], st["cT"], st["e_ps"]
                    negmax = stats.tile([128, 1], F32)
                    nc.vector.tensor_reduce(out=negmax, in_=e_ps, axis=mybir.AxisListType.X,
                                            op=mybir.AluOpType.max, negate=True)
                    p_sb = outp.tile([128, 256], F32)
                    ssum = stats.tile([128, 1], F32)
                    nc.scalar.activation(out=p_sb, in_=e_ps, func=AF.Exp, bias=negmax,
                                         scale=1.0, accum_out=ssum)
                    rsum = stats.tile([128, 1], F32)
                    nc.vector.reciprocal(rsum, ssum)
                    nc.vector.tensor_scalar_mul(p_sb, in0=p_sb, scalar1=rsum)
                    nc.sync.dma_start(out=out_d[b], in_=p_sb)

        for rep in range(reps if not interleave else 0):
            for b in range(BPC):
                # ---- load pre-transposed query/context: [d_part, d_chunk, *] ----
                qryT = loads.tile([128, 2, 128], F32)
                ctxT = loads.tile([128, 2, 256], F32)
                for di in range(2):
                    nc.sync.dma_start(out=qryT[:, di, :], in_=qT_d[b, 128 * di : 128 * (di + 1), :])
                    nc.sync.dma_start(out=ctxT[:, di, :], in_=cT_d[b, 128 * di : 128 * (di + 1), :])

                # ---- projections: qT[h_part, ht, f], cT[h_part, ht, s] ----
                # fp32 copies feed the DVE adds; fp16 copies (for the
                # PE-assisted adds) are written straight from PSUM by ACT.
                qT = proj.tile([128, 2, 128], F32)
                cT = proj.tile([128, 2, 256], F32)
                for ht in range(2):
                    qp = ps_scr.tile([128, 128], F32, tag="tp")
                    for di in range(2):
                        nc.tensor.matmul(
                            qp,
                            lhsT=wqT[:, di, 128 * ht : 128 * (ht + 1)],
                            rhs=qryT[:, di, :],
                            start=(di == 0),
                            stop=(di == 1),
                        )
                    nc.vector.tensor_copy(qT[:, ht, :], qp)
                    cp = ps_scr.tile([128, 256], F32, tag="cp")
                    for di in range(2):
                        nc.tensor.matmul(
                            cp,
                            lhsT=wcT[:, di, 128 * ht : 128 * (ht + 1)],
                            rhs=ctxT[:, di, :],
                            start=(di == 0),
                            stop=(di == 1),
                        )
                    nc.vector.tensor_copy(cT[:, ht, :], cp)

                if assist_s:
                    qT16 = proj.tile([128, 2, 128], F16)
                    cT16 = proj.tile([128, 2, 256], F16)
                    nc.vector.tensor_copy(qT16, qT)
                    nc.vector.tensor_copy(cT16, cT)

                # ---- main loop: E[f, s] accumulates in PSUM ----
                e_ps = ps_e.tile([128, 256], F32)
                s0 = 0
                for bs in s_blocks:
                    if s0 + bs <= assist_s:
                        # adds on PE: A[h, (s, f)] = I@q (bcast s) + I@c (bcast f)
                        t_t = work.tile([128, 2, bs, 128], T_DT, tag="t_t")
                        for ht in range(2):
                            for sq in range(0, bs, 8):
                                a_ps = ps_a.tile([128, 8, 128], F32, tag="a_ps")
                                for half in range(2):
                                    sl4 = slice(4 * half, 4 * half + 4)
                                    nc.tensor.matmul(
                                        a_ps[:, sl4],
                                        lhsT=ident16,
                                        rhs=qT16[:, ht].unsqueeze(1).broadcast_to((128, 4, 128)),
                                        start=True,
                                        stop=False,
                                    )
                                    nc.tensor.matmul(
                                        a_ps[:, sl4],
                                        lhsT=ident16,
                                        rhs=cT16[:, ht, s0 + sq + 4 * half : s0 + sq + 4 * half + 4]
                                        .unsqueeze(2)
                                        .broadcast_to((128, 4, 128)),
                                        start=False,
                                        stop=True,
                                    )
                                nc.scalar.activation(
                                    out=t_t[:, ht, sq : sq + 8], in_=a_ps, func=AF.Tanh
                                )
                    else:
                        a_t = work.tile([128, 2, bs, 128], F32)
                        nc.vector.tensor_add(
                            out=a_t,
                            in0=cT[:, :, s0 : s0 + bs].unsqueeze(3).broadcast_to((128, 2, bs, 128)),
                            in1=qT.unsqueeze(2).broadcast_to((128, 2, bs, 128)),
                        )
                        t_t = work.tile([128, 2, bs, 128], T_DT, tag="t_t")
                        nc.scalar.activation(out=t_t, in_=a_t, func=AF.Tanh)
                    for sl in range(bs):
                        s = s0 + sl
                        for ht in range(2):
                            nc.tensor.matmul(
                                e_ps[:, s : s + 1],
                                lhsT=t_t[:, ht, sl],
                                rhs=v_sb[:, ht : ht + 1],
                                start=(ht == 0),
                                stop=(ht == 1),
                            )
                    s0 += bs
                # tail: ACT-bias route (add fused into tanh, small tiles)
                for s in range(S - n_bias, S):
                    t_b = work.tile([128, 2, 128], T_DT, tag="t_b")
                    for ht in range(2):
                        nc.scalar.activation(
                            out=t_b[:, ht],
                            in_=qT[:, ht, :],
                            func=AF.Tanh,
                            bias=cT[:, ht, s : s + 1],
                        )
                    for ht in range(2):
                        nc.tensor.matmul(
                            e_ps[:, s : s + 1],
                            lhsT=t_b[:, ht],
                            rhs=v_sb[:, ht : ht + 1],
                            start=(ht == 0),
                            stop=(ht == 1),
                        )

                # ---- softmax over s ----
                negmax = stats.tile([128, 1], F32)
                nc.vector.tensor_reduce(
                    out=negmax, in_=e_ps, axis=mybir.AxisListType.X,
                    op=mybir.AluOpType.max, negate=True,
                )
                p_sb = outp.tile([128, 256], F32)
                ssum = stats.tile([128, 1], F32)
                nc.scalar.activation(
                    out=p_sb, in_=e_ps, func=AF.Exp, bias=negmax, scale=1.0, accum_out=ssum,
                )
                rsum = stats.tile([128, 1], F32)
                nc.vector.reciprocal(rsum, ssum)
                nc.vector.tensor_scalar_mul(p_sb, in0=p_sb, scalar1=rsum)
                nc.sync.dma_start(out=out_d[b], in_=p_sb)

    # Walrus allows at most one semaphore wait per engine instruction; Tile
    # can attach several. Split them via event-semaphore joiners.
    import bass_rust

    bass_rust.generate_event_semaphores(nc)
    return nc


def host_prep(query, context, W_q, W_c, v):
    """Transpose inputs so the contraction dim is leading (per core slice)."""
    queryT = np.ascontiguousarray(np.transpose(query, (0, 2, 1)), dtype=np.float32)
    contextT = np.ascontiguousarray(np.transpose(context, (0, 2, 1)), dtype=np.float32)
    w_qT = np.ascontiguousarray(np.transpose(W_q), dtype=np.float32)
    w_cT = np.ascontiguousarray(np.transpose(W_c), dtype=np.float32)
    v2 = np.ascontiguousarray(v, dtype=np.float32).reshape(H, 1)
    return queryT, contextT, w_qT, w_cT, v2


_RUNNER_CACHE = None


def _make_runner():
    """Compile the program once; return f(concat_inputs) -> concat out."""
    import jax
    from jax.sharding import Mesh, PartitionSpec
    from jax.experimental.shard_map import shard_map
    from concourse import bass2jax

    nc = build_program()
    bass2jax.install_neuronx_cc_hook()
    partition_name = nc.partition_id_tensor.name if nc.partition_id_tensor else None
    in_names, out_names, out_avals = [], [], []
    for alloc in nc.m.functions[0].allocations:
        if not isinstance(alloc, mybir.MemoryLocationSet):
            continue
        name = alloc.memorylocations[0].name
        if alloc.kind == "ExternalInput":
            if name != partition_name:
                in_names.append(name)
        elif alloc.kind == "ExternalOutput":
            out_names.append(name)
            out_avals.append(
                jax.core.ShapedArray(tuple(alloc.tensor_shape), mybir.dt.np(alloc.dtype))
            )
    n_params = len(in_names)
    all_in_names = list(in_names) + out_names
    if partition_name is not None:
        all_in_names.append(partition_name)

    def _body(*args):
        operands = list(args)
        if partition_name is not None:
            operands.append(bass2jax.partition_id_tensor())
        return tuple(
            bass2jax._bass_exec_p.bind(
                *operands,
                out_avals=tuple(out_avals),
                in_names=tuple(all_in_names),
                out_names=tuple(out_names),
                lowering_input_output_aliases=(),
                sim_require_finite=True,
                sim_require_nnan=True,
                nc=nc,
            )
        )

    devices = jax.devices()[:NCORES]
    mesh = Mesh(np.asarray(devices), ("core",))
    n_outs = len(out_names)
    sharded = jax.jit(
        shard_map(
            _body,
            mesh=mesh,
            in_specs=(PartitionSpec("core"),) * (n_params + n_outs),
            out_specs=(PartitionSpec("core"),) * n_outs,
            check_rep=False,
        ),
        keep_unused=True,
    )
    zeros = [np.zeros((NCORES * a.shape[0], *a.shape[1:]), a.dtype) for a in out_avals]
    oi = out_names.index("out")

    def run(by_name: dict):
        args = [by_name[n] for n in in_names] + zeros
        out = sharded(*args)
        return np.asarray(out[oi])

    return run


def kernel(**inputs: np.ndarray) -> np.ndarray:
    global _RUNNER_CACHE
    queryT, contextT, w_qT, w_cT, v2 = host_prep(
        inputs["query"], inputs["context"], inputs["W_q"], inputs["W_c"], inputs["v"]
    )
    if _RUNNER_CACHE is None:
        _RUNNER_CACHE = _make_runner()
    out = _RUNNER_CACHE(
        {
            "queryT": queryT.reshape(B, D, F),
            "contextT": contextT.reshape(B, D, S),
            "w_qT": np.broadcast_to(w_qT, (NCORES, D, H)).reshape(NCORES * D, H),
            "w_cT": np.broadcast_to(w_cT, (NCORES, D, H)).reshape(NCORES * D, H),
            "v": np.broadcast_to(v2, (NCORES, H, 1)).reshape(NCORES * H, 1),
        }
    )
    return np.ascontiguousarray(out.reshape(B, F, S).astype(np.float32))


if __name__ == "__main__":
    rng = np.random.default_rng(0)
    ins = {
        "query": rng.standard_normal((B, F, D), dtype=np.float32),
        "context": rng.standard_normal((B, S, D), dtype=np.float32),
        "W_q": rng.standard_normal((H, D), dtype=np.float32) / np.sqrt(D),
        "W_c": rng.standard_normal((H, D), dtype=np.float32) / np.sqrt(D),
        "v": rng.standard_normal((H,), dtype=np.float32),
    }
    o = kernel(**ins)
    print(o.shape, o.dtype, o.sum())



# revision 10
# speedup vs baseline: 2.0381x; 2.0381x over previous
"""Bahdanau-style additive attention kernel for Trainium2 (8 NeuronCores).

Computes, per batch b:
    q = query[b] @ W_q.T            # [F, H]
    c = context[b] @ W_c.T          # [S, H]
    E[f, s] = sum_h v[h] * tanh(q[f, h] + c[s, h])
    out[b] = softmax(E, axis=-1)    # [F, S]

Key idea: replace the elementwise tanh over F*S*H (ACT-bound, ~110us/core)
with a separable Fourier expansion

    tanh(t) ~ sum_k b_k sin(pi k t / L),   t = q + c
    sin(w_k(q+c)) = sin_k(q)cos_k(c) + cos_k(q)sin_k(c)

so the F*S*H work becomes PE matmuls over h, and transcendentals only touch
the [F,H]/[S,H] projections. Harmonic set {1..8,10,12,14,16,20}:

  - only two ACT Sins: s1 = sin(2*pi*xt), sh = sin(pi*xt) (args in range).
  - cosines via multiple-angle identities (ACT Square + cheap TS):
      c2 = 1-2*s1^2, c3 = c1*(1-4*s1^2), c4 = 1-8*st2^2, ...
  - sines by doubling products (fp16 TT): st2 = s1*c1 (= sin2/2),
    st4 = st2*c2 (= sin4/4), st6 = s3*c3, st8 = st4*c4, st10 = s5*c5;
    s7/c7 by angle addition 3+4. Scale factors lam fold into coefficients.
  - even harmonics {12,14,16,20} = 2*{6,7,8,10} enter as products
    P = st_a*c_a, Q = st_a^2 with the expansion
      b_m sin_m(q+c) = 2P_q(1-2Q_c)+(1-2Q_q)2P_c  (f-only term dropped:
    softmax-invariant; s-only term via an all-(v*2*b*lam) stationary).

Coefficients fit offline by row-centered least squares on energy residuals.
Sharding: data-parallel over batch. 16 batches -> 8 cores x 2 batches.
Softmax: DVE reduce_max(negate) -> ACT Exp(accum_out) -> reciprocal+scale.
"""

import math
import sys

for _p in ("/opt/trn_rl_repo", "/opt/pypackages"):
    if _p not in sys.path:
        sys.path.append(_p)

from contextlib import ExitStack

import numpy as np

import concourse.bass as bass
import concourse.tile as tile
from concourse import mybir

B, F, S, D, H = 16, 128, 256, 256, 256
NCORES = 8
BPC = B // NCORES  # batches per core

L_PERIOD = 12.4
# harmonic -> (coefficient b, lam scale of the stored sine feature)
STD_KS = [1, 2, 3, 4, 5, 6, 7, 8, 10]
EVEN_KS = [12, 14, 16, 20]   # m -> half harmonic a=m//2 in STD_KS
B_COEF = {
    1: 1.2388846116e+00, 2: 3.3479903211e-03, 3: 3.3120957792e-01,
    4: 1.1610640847e-02, 5: 1.2579096501e-01, 6: 1.8345634100e-02,
    7: 3.9961303841e-02, 8: 2.7940886524e-02, 10: 2.1276806991e-02,
    12: 7.9695779625e-03, 14: 3.6958586635e-03, 16: 2.3388407203e-03,
    20: 8.9165962593e-04,
}
LAM = {1: 1.0, 2: 2.0, 3: 1.0, 4: 4.0, 5: 1.0, 6: 2.0, 7: 1.0, 8: 8.0, 10: 2.0}

F16 = mybir.dt.float16
F32 = mybir.dt.float32
AF = mybir.ActivationFunctionType
ALU = mybir.AluOpType

SLAB = 2 * (F + S)          # per-batch slab columns: 2 ht * (f 128 | s 256)
NX = BPC * SLAB             # X slab free dim (both batches): 1536
TWO_PI = 2.0 * math.pi


def _off(b, ht):
    return b * SLAB + ht * (F + S)


def build_program(reps: int = 1, s_blocks=None, assist_s=None, interleave=False) -> bass.Bass:
    nc = bass.Bass()
    qT_d = nc.declare_dram_parameter("queryT", [BPC, D, F], F32, isOutput=False)
    cT_d = nc.declare_dram_parameter("contextT", [BPC, D, S], F32, isOutput=False)
    wqT_d = nc.declare_dram_parameter("w_qT", [D, H], F32, isOutput=False)
    wcT_d = nc.declare_dram_parameter("w_cT", [D, H], F32, isOutput=False)
    v_d = nc.declare_dram_parameter("v", [H, 1], F32, isOutput=False)
    out_d = nc.declare_dram_parameter("out", [BPC, F, S], F32, isOutput=True)

    with tile.TileContext(nc) as tc, ExitStack() as ctx:
        consts = ctx.enter_context(tc.tile_pool(name="consts", bufs=1))
        loads = ctx.enter_context(tc.tile_pool(name="loads", bufs=2))
        xpool = ctx.enter_context(tc.tile_pool(name="xpool", bufs=2))
        upool = ctx.enter_context(tc.tile_pool(name="upool", bufs=3))
        fpool = ctx.enter_context(tc.tile_pool(name="fpool", bufs=1))
        tpool = ctx.enter_context(tc.tile_pool(name="tpool", bufs=5))
        scpool = ctx.enter_context(tc.tile_pool(name="scpool", bufs=6))
        dpool = ctx.enter_context(tc.tile_pool(name="dpool", bufs=2))
        stats = ctx.enter_context(tc.tile_pool(name="stats", bufs=4))
        outp = ctx.enter_context(tc.tile_pool(name="outp", bufs=2))
        ps_scr = ctx.enter_context(tc.tile_pool(name="ps_scr", bufs=2, space="PSUM"))
        ps_e = ctx.enter_context(tc.tile_pool(name="ps_e", bufs=2, space="PSUM"))

        v32 = consts.tile([128, 2], F32)
        for ht in range(2):
            nc.sync.dma_start(out=v32[:, ht : ht + 1], in_=v_d[128 * ht : 128 * (ht + 1), :])

        # W^T tiles (pre-scaled by 1/(2L) on host): [d_part, d_chunk, h]
        wqT = consts.tile([128, 2, 256], F32)
        wcT = consts.tile([128, 2, 256], F32)
        for di in range(2):
            nc.sync.dma_start(out=wqT[:, di, :], in_=wqT_d[128 * di : 128 * (di + 1), :])
            nc.sync.dma_start(out=wcT[:, di, :], in_=wcT_d[128 * di : 128 * (di + 1), :])

        for rep in range(reps):
            # ---- load + project both batches; X = proj/(2L) in SBUF ----
            X = xpool.tile([128, NX], F32)
            e_ps = []
            for b in range(BPC):
                e_b = ps_e.tile([128, S], F32, tag=f"e{b}")
                e_ps.append(e_b)
            for b in range(BPC):
                qryT = loads.tile([128, 2, F], F32, tag="qry")
                ctxT = loads.tile([128, 2, S], F32, tag="ctx")
                for di in range(2):
                    nc.sync.dma_start(out=qryT[:, di, :], in_=qT_d[b, 128 * di : 128 * (di + 1), :])
                    nc.sync.dma_start(out=ctxT[:, di, :], in_=cT_d[b, 128 * di : 128 * (di + 1), :])
                for ht in range(2):
                    off = _off(b, ht)
                    qp = ps_scr.tile([128, F], F32, tag="qp")
                    for di in range(2):
                        nc.tensor.matmul(qp, lhsT=wqT[:, di, 128 * ht : 128 * (ht + 1)],
                                         rhs=qryT[:, di, :], start=(di == 0), stop=(di == 1))
                    nc.scalar.activation(out=X[:, off : off + F], in_=qp, func=AF.Copy, bias=0.0)
                    cp = ps_scr.tile([128, S], F32, tag="cp")
                    for di in range(2):
                        nc.tensor.matmul(cp, lhsT=wcT[:, di, 128 * ht : 128 * (ht + 1)],
                                         rhs=ctxT[:, di, :], start=(di == 0), stop=(di == 1))
                    nc.scalar.activation(out=X[:, off + F : off + F + S], in_=cp, func=AF.Copy, bias=0.0)

            # ---- materialized harmonic features F[k] = [128, 2(s|c), NX] fp16 ----
            Ft = {}
            for k in STD_KS:
                Ft[k] = fpool.tile([128, 2, NX], F16, tag=f"f{k}", name=f"feat{k}")

            # ---- features from 2 ACT Sins + polynomial identities ----
            # s1 = sin(2*pi*xt), sh = sin(pi*xt); everything else derived.
            nc.scalar.activation(out=Ft[1][:, 0], in_=X, func=AF.Sin, scale=TWO_PI)
            sh = tpool.tile([128, NX], F16, tag="t", name="sh")
            nc.scalar.activation(out=sh, in_=X, func=AF.Sin, scale=math.pi)
            Qh = tpool.tile([128, NX], F16, tag="t", name="Qh")
            nc.scalar.activation(out=Qh, in_=sh, func=AF.Square)
            nc.vector.tensor_scalar(out=Ft[1][:, 1], in0=Qh, scalar1=-2.0, scalar2=1.0,
                                    op0=ALU.mult, op1=ALU.add)          # c1
            Q1 = tpool.tile([128, NX], F16, tag="q1", name="Q1")
            nc.scalar.activation(out=Q1, in_=Ft[1][:, 0], func=AF.Square)
            nc.vector.tensor_scalar(out=Ft[2][:, 1], in0=Q1, scalar1=-2.0, scalar2=1.0,
                                    op0=ALU.mult, op1=ALU.add)          # c2
            nc.vector.tensor_tensor(out=Ft[2][:, 0], in0=Ft[1][:, 0], in1=Ft[1][:, 1],
                                    op=ALU.mult)                        # st2 = sin2/2
            w3 = tpool.tile([128, NX], F16, tag="t", name="w3")
            nc.vector.tensor_scalar(out=w3, in0=Q1, scalar1=-4.0, scalar2=3.0,
                                    op0=ALU.mult, op1=ALU.add)
            nc.vector.tensor_tensor(out=Ft[3][:, 0], in0=Ft[1][:, 0], in1=w3,
                                    op=ALU.mult)                        # s3 = s1(3-4Q1)
            w3c = tpool.tile([128, NX], F16, tag="t", name="w3c")
            nc.vector.tensor_scalar(out=w3c, in0=Q1, scalar1=-4.0, scalar2=1.0,
                                    op0=ALU.mult, op1=ALU.add)
            nc.vector.tensor_tensor(out=Ft[3][:, 1], in0=Ft[1][:, 1], in1=w3c,
                                    op=ALU.mult)                        # c3 = c1(1-4Q1)
            Q2 = tpool.tile([128, NX], F16, tag="t", name="Q2")
            nc.scalar.activation(out=Q2, in_=Ft[2][:, 0], func=AF.Square)
            nc.vector.tensor_scalar(out=Ft[4][:, 1], in0=Q2, scalar1=-8.0, scalar2=1.0,
                                    op0=ALU.mult, op1=ALU.add)          # c4 = 1-8*st2^2
            nc.vector.tensor_tensor(out=Ft[4][:, 0], in0=Ft[2][:, 0], in1=Ft[2][:, 1],
                                    op=ALU.mult)                        # st4 = sin4/4
            t47 = tpool.tile([128, NX], F16, tag="t47", name="t47")
            nc.vector.tensor_scalar_mul(out=t47, in0=Ft[4][:, 0], scalar1=4.0)  # sin4
            # s5/c5 = angle addition 1+4; s7/c7 = 3+4 (sharing c4, t47)
            tA = tpool.tile([128, NX], F16, tag="t", name="tA")
            tB = tpool.tile([128, NX], F16, tag="t", name="tB")
            nc.vector.tensor_tensor(out=tA, in0=Ft[1][:, 0], in1=Ft[4][:, 1], op=ALU.mult)
            nc.vector.tensor_tensor(out=tB, in0=Ft[1][:, 1], in1=t47, op=ALU.mult)
            nc.vector.tensor_tensor(out=Ft[5][:, 0], in0=tA, in1=tB, op=ALU.add)   # s5
            tC = tpool.tile([128, NX], F16, tag="t", name="tC")
            tD = tpool.tile([128, NX], F16, tag="t", name="tD")
            nc.vector.tensor_tensor(out=tC, in0=Ft[1][:, 1], in1=Ft[4][:, 1], op=ALU.mult)
            nc.vector.tensor_tensor(out=tD, in0=Ft[1][:, 0], in1=t47, op=ALU.mult)
            nc.vector.tensor_tensor(out=Ft[5][:, 1], in0=tC, in1=tD, op=ALU.subtract)  # c5
            nc.vector.tensor_tensor(out=Ft[6][:, 0], in0=Ft[3][:, 0], in1=Ft[3][:, 1],
                                    op=ALU.mult)                        # st6 = sin6/2
            Q3 = tpool.tile([128, NX], F16, tag="t", name="Q3")
            nc.scalar.activation(out=Q3, in_=Ft[3][:, 0], func=AF.Square)
            nc.vector.tensor_scalar(out=Ft[6][:, 1], in0=Q3, scalar1=-2.0, scalar2=1.0,
                                    op0=ALU.mult, op1=ALU.add)          # c6
            tE = tpool.tile([128, NX], F16, tag="t", name="tE")
            tF = tpool.tile([128, NX], F16, tag="t", name="tF")
            nc.vector.tensor_tensor(out=tE, in0=Ft[3][:, 0], in1=Ft[4][:, 1], op=ALU.mult)
            nc.vector.tensor_tensor(out=tF, in0=Ft[3][:, 1], in1=t47, op=ALU.mult)
            nc.vector.tensor_tensor(out=Ft[7][:, 0], in0=tE, in1=tF, op=ALU.add)   # s7
            tG = tpool.tile([128, NX], F16, tag="t", name="tG")
            tH = tpool.tile([128, NX], F16, tag="t", name="tH")
            nc.vector.tensor_tensor(out=tG, in0=Ft[3][:, 1], in1=Ft[4][:, 1], op=ALU.mult)
            nc.vector.tensor_tensor(out=tH, in0=Ft[3][:, 0], in1=t47, op=ALU.mult)
            nc.vector.tensor_tensor(out=Ft[7][:, 1], in0=tG, in1=tH, op=ALU.subtract)  # c7
            nc.vector.tensor_tensor(out=Ft[8][:, 0], in0=Ft[4][:, 0], in1=Ft[4][:, 1],
                                    op=ALU.mult)                        # st8 = sin8/8
            Q4 = tpool.tile([128, NX], F16, tag="t", name="Q4")
            nc.scalar.activation(out=Q4, in_=Ft[4][:, 0], func=AF.Square)
            nc.vector.tensor_scalar(out=Ft[8][:, 1], in0=Q4, scalar1=-32.0, scalar2=1.0,
                                    op0=ALU.mult, op1=ALU.add)          # c8
            nc.vector.tensor_tensor(out=Ft[10][:, 0], in0=Ft[5][:, 0], in1=Ft[5][:, 1],
                                    op=ALU.mult)                        # st10 = sin10/2
            Q5 = tpool.tile([128, NX], F16, tag="t", name="Q5")
            nc.scalar.activation(out=Q5, in_=Ft[5][:, 0], func=AF.Square)
            nc.vector.tensor_scalar(out=Ft[10][:, 1], in0=Q5, scalar1=-2.0, scalar2=1.0,
                                    op0=ALU.mult, op1=ALU.add)          # c10

            # ---- energy accumulation ----
            started = [False] * BPC

            def mm(b, lhsT, rhs, last=False):
                nc.tensor.matmul(e_ps[b], lhsT=lhsT, rhs=rhs,
                                 start=not started[b], stop=last)
                started[b] = True

            for k in STD_KS:
                bl = float(B_COEF[k] * LAM[k])
                for b in range(BPC):
                    for ht in range(2):
                        off = _off(b, ht)
                        sc = scpool.tile([128, 2, F], F16, tag="sc")
                        nc.vector.tensor_scalar(
                            out=sc, in0=Ft[k][:, :, off : off + F],
                            scalar1=v32[:, ht : ht + 1], scalar2=bl,
                            op0=ALU.mult, op1=ALU.mult)
                        coff = off + F
                        mm(b, sc[:, 0, :], Ft[k][:, 1, coff : coff + S])
                        mm(b, sc[:, 1, :], Ft[k][:, 0, coff : coff + S])

            n_ev = len(EVEN_KS)
            for mi, m in enumerate(EVEN_KS):
                a = m // 2
                bm = float(B_COEF[m])
                lam = LAM[a]
                last_ev = mi == n_ev - 1
                Pd = dpool.tile([128, NX], F16, tag="pd")
                nc.vector.tensor_tensor(out=Pd, in0=Ft[a][:, 0], in1=Ft[a][:, 1], op=ALU.mult)
                Qd = dpool.tile([128, NX], F16, tag="qd")
                nc.scalar.activation(out=Qd, in_=Ft[a][:, 0], func=AF.Square)
                c_t2 = float(-4.0 * bm * lam ** 3)
                c_t3 = float(2.0 * bm * lam)
                for ht in range(2):
                    ones_v = scpool.tile([128, F], F16, tag="ov")
                    nc.vector.tensor_scalar(
                        out=ones_v, in0=v32[:, ht : ht + 1].broadcast_to((128, F)),
                        scalar1=c_t3, scalar2=None, op0=ALU.mult, op1=ALU.bypass)
                    for b in range(BPC):
                        off = _off(b, ht)
                        coff = off + F
                        st2 = scpool.tile([128, F], F16, tag="st2")
                        st4 = scpool.tile([128, F], F16, tag="st4")
                        nc.vector.tensor_scalar(
                            out=st2, in0=Pd[:, off : off + F],
                            scalar1=v32[:, ht : ht + 1], scalar2=c_t2,
                            op0=ALU.mult, op1=ALU.mult)
                        nc.vector.tensor_scalar(
                            out=st4, in0=Qd[:, off : off + F],
                            scalar1=v32[:, ht : ht + 1], scalar2=c_t2,
                            op0=ALU.mult, op1=ALU.mult)
                        mm(b, st2, Qd[:, coff : coff + S])
                        mm(b, st4, Pd[:, coff : coff + S])
                        mm(b, ones_v, Pd[:, coff : coff + S], last=last_ev and ht == 1)

            # ---- softmax over s ----
            for b in range(BPC):
                negmax = stats.tile([128, 1], F32)
                nc.vector.tensor_reduce(out=negmax, in_=e_ps[b], axis=mybir.AxisListType.X,
                                        op=ALU.max, negate=True)
                p_sb = outp.tile([128, S], F32)
                ssum = stats.tile([128, 1], F32)
                nc.scalar.activation(out=p_sb, in_=e_ps[b], func=AF.Exp, bias=negmax,
                                     scale=1.0, accum_out=ssum)
                rsum = stats.tile([128, 1], F32)
                nc.vector.reciprocal(rsum, ssum)
                nc.vector.tensor_scalar_mul(p_sb, in0=p_sb, scalar1=rsum)
                nc.sync.dma_start(out=out_d[b], in_=p_sb)

    import bass_rust

    bass_rust.generate_event_semaphores(nc)
    return nc


def host_prep(query, context, W_q, W_c, v):
    """Transpose inputs; W is pre-scaled by 1/(2L) so projections emit
    phase-period units directly."""
    s = 1.0 / (2.0 * L_PERIOD)
    queryT = np.ascontiguousarray(np.transpose(query, (0, 2, 1)), dtype=np.float32)
    contextT = np.ascontiguousarray(np.transpose(context, (0, 2, 1)), dtype=np.float32)
    w_qT = np.ascontiguousarray(np.transpose(W_q) * s, dtype=np.float32)
    w_cT = np.ascontiguousarray(np.transpose(W_c) * s, dtype=np.float32)
    v2 = np.ascontiguousarray(v, dtype=np.float32).reshape(H, 1)
    return queryT, contextT, w_qT, w_cT, v2


_RUNNER_CACHE = None


def _make_runner():
    """Compile the program once; return f(concat_inputs) -> concat out."""
    import jax
    from jax.sharding import Mesh, PartitionSpec
    from jax.experimental.shard_map import shard_map
    from concourse import bass2jax

    nc = build_program()
    bass2jax.install_neuronx_cc_hook()
    partition_name = nc.partition_id_tensor.name if nc.partition_id_tensor else None
    in_names, out_names, out_avals = [], [], []
    for alloc in nc.m.functions[0].allocations:
        if not isinstance(alloc, mybir.MemoryLocationSet):
            continue
        name = alloc.memorylocations[0].name
        if alloc.kind == "ExternalInput":
            if name != partition_name:
                in_names.append(name)
        elif alloc.kind == "ExternalOutput":
            out_names.append(name)
            out_avals.append(
                jax.core.ShapedArray(tuple(alloc.tensor_shape), mybir.dt.np(alloc.dtype))
            )
    n_params = len(in_names)
    all_in_names = list(in_names) + out_names
    if partition_name is not None:
        all_in_names.append(partition_name)

    def _body(*args):
        operands = list(args)
        if partition_name is not None:
            operands.append(bass2jax.partition_id_tensor())
        return tuple(
            bass2jax._bass_exec_p.bind(
                *operands,
                out_avals=tuple(out_avals),
                in_names=tuple(all_in_names),
                out_names=tuple(out_names),
                lowering_input_output_aliases=(),
                sim_require_finite=True,
                sim_require_nnan=True,
                nc=nc,
            )
        )

    devices = jax.devices()[:NCORES]
    mesh = Mesh(np.asarray(devices), ("core",))
    n_outs = len(out_names)
    sharded = jax.jit(
        shard_map(
            _body,
            mesh=mesh,
            in_specs=(PartitionSpec("core"),) * (n_params + n_outs),
            out_specs=(PartitionSpec("core"),) * n_outs,
            check_rep=False,
        ),
        keep_unused=True,
    )
    zeros = [np.zeros((NCORES * a.shape[0], *a.shape[1:]), a.dtype) for a in out_avals]
    oi = out_names.index("out")

    def run(by_name: dict):
        args = [by_name[n] for n in in_names] + zeros
        out = sharded(*args)
        return np.asarray(out[oi])

    return run


def kernel(**inputs: np.ndarray) -> np.ndarray:
    global _RUNNER_CACHE
    queryT, contextT, w_qT, w_cT, v2 = host_prep(
        inputs["query"], inputs["context"], inputs["W_q"], inputs["W_c"], inputs["v"]
    )
    if _RUNNER_CACHE is None:
        _RUNNER_CACHE = _make_runner()
    out = _RUNNER_CACHE(
        {
            "queryT": queryT.reshape(B, D, F),
            "contextT": contextT.reshape(B, D, S),
            "w_qT": np.broadcast_to(w_qT, (NCORES, D, H)).reshape(NCORES * D, H),
            "w_cT": np.broadcast_to(w_cT, (NCORES, D, H)).reshape(NCORES * D, H),
            "v": np.broadcast_to(v2, (NCORES, H, 1)).reshape(NCORES * H, 1),
        }
    )
    return np.ascontiguousarray(out.reshape(B, F, S).astype(np.float32))


if __name__ == "__main__":
    rng = np.random.default_rng(0)
    ins = {
        "query": rng.standard_normal((B, F, D), dtype=np.float32),
        "context": rng.standard_normal((B, S, D), dtype=np.float32),
        "W_q": rng.standard_normal((H, D), dtype=np.float32) / np.sqrt(D),
        "W_c": rng.standard_normal((H, D), dtype=np.float32) / np.sqrt(D),
        "v": rng.standard_normal((H,), dtype=np.float32),
    }
    o = kernel(**ins)
    print(o.shape, o.dtype, o.sum())


# revision 11
# speedup vs baseline: 16.5965x; 8.1430x over previous
"""Bahdanau-style additive attention kernel for Trainium2 (8 NeuronCores).

Computes, per batch b:
    q = query[b] @ W_q.T            # [F, H]
    c = context[b] @ W_c.T          # [S, H]
    E[f, s] = sum_h v[h] * tanh(q[f, h] + c[s, h])
    out[b] = softmax(E, axis=-1)    # [F, S]

Key idea: replace the elementwise tanh over F*S*H (ACT-bound, ~110us/core)
with a separable Fourier expansion

    tanh(t) ~ sum_k b_k sin(pi k t / L),   t = q + c
    sin(w_k(q+c)) = sin_k(q)cos_k(c) + cos_k(q)sin_k(c)

so the F*S*H work becomes PE matmuls over h, and transcendentals only touch
the [F,H]/[S,H] projections. Harmonic set {1..8,10,12,14,16,20}:

  - only two ACT Sins: s1 = sin(2*pi*xt), sh = sin(pi*xt) (args in range).
  - cosines via multiple-angle identities (ACT Square + cheap TS):
      c2 = 1-2*s1^2, c3 = c1*(1-4*s1^2), c4 = 1-8*st2^2, ...
  - sines by doubling products (fp16 TT): st2 = s1*c1 (= sin2/2),
    st4 = st2*c2 (= sin4/4), st6 = s3*c3, st8 = st4*c4, st10 = s5*c5;
    s7/c7 by angle addition 3+4. Scale factors lam fold into coefficients.
  - even harmonics {12,14,16,20} = 2*{6,7,8,10} enter as products
    P = st_a*c_a, Q = st_a^2 with the expansion
      b_m sin_m(q+c) = 2P_q(1-2Q_c)+(1-2Q_q)2P_c  (f-only term dropped:
    softmax-invariant; s-only term via an all-(v*2*b*lam) stationary).

Coefficients fit offline by row-centered least squares on energy residuals.
Sharding: data-parallel over batch. 16 batches -> 8 cores x 2 batches.
Softmax: DVE reduce_max(negate) -> ACT Exp(accum_out) -> reciprocal+scale.
"""

import math
import sys

for _p in ("/opt/trn_rl_repo", "/opt/pypackages"):
    if _p not in sys.path:
        sys.path.append(_p)

from contextlib import ExitStack

import numpy as np

import concourse.bass as bass
import concourse.tile as tile
from concourse import mybir

B, F, S, D, H = 16, 128, 256, 256, 256
NCORES = 8
BPC = B // NCORES  # batches per core

L_PERIOD = 12.4
# harmonic -> (coefficient b, lam scale of the stored sine feature)
STD_KS = [1, 2, 3, 4, 5, 6, 7, 8, 10]
EVEN_KS = [12, 14, 16, 20]   # m -> half harmonic a=m//2 in STD_KS
B_COEF = {
    1: 1.2388846116e+00, 2: 3.3479903211e-03, 3: 3.3120957792e-01,
    4: 1.1610640847e-02, 5: 1.2579096501e-01, 6: 1.8345634100e-02,
    7: 3.9961303841e-02, 8: 2.7940886524e-02, 10: 2.1276806991e-02,
    12: 7.9695779625e-03, 14: 3.6958586635e-03, 16: 2.3388407203e-03,
    20: 8.9165962593e-04,
}
LAM = {1: 1.0, 2: 2.0, 3: 1.0, 4: 4.0, 5: 1.0, 6: 2.0, 7: 1.0, 8: 8.0, 10: 2.0}

F16 = mybir.dt.float16
F32 = mybir.dt.float32
AF = mybir.ActivationFunctionType
ALU = mybir.AluOpType

SLAB = 2 * (F + S)          # per-batch slab columns: 2 ht * (f 128 | s 256)
NX = BPC * SLAB             # X slab free dim (both batches): 1536
TWO_PI = 2.0 * math.pi


def _off(b, ht):
    return b * SLAB + ht * (F + S)


def build_program(reps: int = 1, s_blocks=None, assist_s=None, interleave=False) -> bass.Bass:
    nc = bass.Bass()
    qT_d = nc.declare_dram_parameter("queryT", [BPC, D, F], F32, isOutput=False)
    cT_d = nc.declare_dram_parameter("contextT", [BPC, D, S], F32, isOutput=False)
    wqT_d = nc.declare_dram_parameter("w_qT", [D, H], F32, isOutput=False)
    wcT_d = nc.declare_dram_parameter("w_cT", [D, H], F32, isOutput=False)
    v_d = nc.declare_dram_parameter("v", [H, 1], F32, isOutput=False)
    out_d = nc.declare_dram_parameter("out", [BPC, F, S], F32, isOutput=True)

    with tile.TileContext(nc) as tc, ExitStack() as ctx:
        consts = ctx.enter_context(tc.tile_pool(name="consts", bufs=1))
        loads = ctx.enter_context(tc.tile_pool(name="loads", bufs=2))
        upool = ctx.enter_context(tc.tile_pool(name="upool", bufs=3))
        fpool = ctx.enter_context(tc.tile_pool(name="fpool", bufs=2))
        tpool = ctx.enter_context(tc.tile_pool(name="tpool", bufs=5))
        scpool = ctx.enter_context(tc.tile_pool(name="scpool", bufs=6))
        dpool = ctx.enter_context(tc.tile_pool(name="dpool", bufs=2))
        stats = ctx.enter_context(tc.tile_pool(name="stats", bufs=4))
        outp = ctx.enter_context(tc.tile_pool(name="outp", bufs=2))
        ps_scr = ctx.enter_context(tc.tile_pool(name="ps_scr", bufs=2, space="PSUM"))
        ps_e = ctx.enter_context(tc.tile_pool(name="ps_e", bufs=2, space="PSUM"))

        v32 = consts.tile([128, 2], F32)
        for ht in range(2):
            nc.sync.dma_start(out=v32[:, ht : ht + 1], in_=v_d[128 * ht : 128 * (ht + 1), :])

        # W^T tiles (pre-scaled by 1/(2L) on host): [d_part, d_chunk, h]
        wqT = consts.tile([128, 2, 256], F32)
        wcT = consts.tile([128, 2, 256], F32)
        for di in range(2):
            nc.sync.dma_start(out=wqT[:, di, :], in_=wqT_d[128 * di : 128 * (di + 1), :])
            nc.sync.dma_start(out=wcT[:, di, :], in_=wcT_d[128 * di : 128 * (di + 1), :])

        for rep in range(reps):
            # ---- load + project both batches; Sin pieces read PSUM directly ----
            e_ps = []
            for b in range(BPC):
                e_b = ps_e.tile([128, S], F32, tag=f"e{b}")
                e_ps.append(e_b)
            Ft = {}
            for k in STD_KS:
                Ft[k] = fpool.tile([128, 2, NX], F16, tag=f"f{k}", name=f"feat{k}")
            sh = tpool.tile([128, NX], F16, tag="sh", name="sh")
            for b in range(BPC):
                qryT = loads.tile([128, 2, F], F32, tag="qry")
                ctxT = loads.tile([128, 2, S], F32, tag="ctx")
                for di in range(2):
                    nc.sync.dma_start(out=qryT[:, di, :], in_=qT_d[b, 128 * di : 128 * (di + 1), :])
                    nc.sync.dma_start(out=ctxT[:, di, :], in_=cT_d[b, 128 * di : 128 * (di + 1), :])
                for ht in range(2):
                    off = _off(b, ht)
                    qp = ps_scr.tile([128, F], F32, tag="qp")
                    for di in range(2):
                        nc.tensor.matmul(qp, lhsT=wqT[:, di, 128 * ht : 128 * (ht + 1)],
                                         rhs=qryT[:, di, :], start=(di == 0), stop=(di == 1))
                    nc.scalar.activation(out=Ft[1][:, 0, off : off + F], in_=qp,
                                         func=AF.Sin, scale=TWO_PI)
                    nc.scalar.activation(out=sh[:, off : off + F], in_=qp,
                                         func=AF.Sin, scale=math.pi)
                    cp = ps_scr.tile([128, S], F32, tag="cp")
                    for di in range(2):
                        nc.tensor.matmul(cp, lhsT=wcT[:, di, 128 * ht : 128 * (ht + 1)],
                                         rhs=ctxT[:, di, :], start=(di == 0), stop=(di == 1))
                    nc.scalar.activation(out=Ft[1][:, 0, off + F : off + F + S], in_=cp,
                                         func=AF.Sin, scale=TWO_PI)
                    nc.scalar.activation(out=sh[:, off + F : off + F + S], in_=cp,
                                         func=AF.Sin, scale=math.pi)
            Qh = tpool.tile([128, NX], F16, tag="t", name="Qh")
            nc.scalar.activation(out=Qh, in_=sh, func=AF.Square)
            nc.vector.tensor_scalar(out=Ft[1][:, 1], in0=Qh, scalar1=-2.0, scalar2=1.0,
                                    op0=ALU.mult, op1=ALU.add)          # c1
            Q1 = tpool.tile([128, NX], F16, tag="q1", name="Q1")
            nc.scalar.activation(out=Q1, in_=Ft[1][:, 0], func=AF.Square)
            nc.vector.tensor_scalar(out=Ft[2][:, 1], in0=Q1, scalar1=-2.0, scalar2=1.0,
                                    op0=ALU.mult, op1=ALU.add)          # c2
            nc.vector.tensor_tensor(out=Ft[2][:, 0], in0=Ft[1][:, 0], in1=Ft[1][:, 1],
                                    op=ALU.mult)                        # st2 = sin2/2
            w3 = tpool.tile([128, NX], F16, tag="t", name="w3")
            nc.vector.tensor_scalar(out=w3, in0=Q1, scalar1=-4.0, scalar2=3.0,
                                    op0=ALU.mult, op1=ALU.add)
            nc.vector.tensor_tensor(out=Ft[3][:, 0], in0=Ft[1][:, 0], in1=w3,
                                    op=ALU.mult)                        # s3 = s1(3-4Q1)
            w3c = tpool.tile([128, NX], F16, tag="t", name="w3c")
            nc.vector.tensor_scalar(out=w3c, in0=Q1, scalar1=-4.0, scalar2=1.0,
                                    op0=ALU.mult, op1=ALU.add)
            nc.vector.tensor_tensor(out=Ft[3][:, 1], in0=Ft[1][:, 1], in1=w3c,
                                    op=ALU.mult)                        # c3 = c1(1-4Q1)
            Q2 = tpool.tile([128, NX], F16, tag="t", name="Q2")
            nc.scalar.activation(out=Q2, in_=Ft[2][:, 0], func=AF.Square)
            nc.vector.tensor_scalar(out=Ft[4][:, 1], in0=Q2, scalar1=-8.0, scalar2=1.0,
                                    op0=ALU.mult, op1=ALU.add)          # c4 = 1-8*st2^2
            nc.vector.tensor_tensor(out=Ft[4][:, 0], in0=Ft[2][:, 0], in1=Ft[2][:, 1],
                                    op=ALU.mult)                        # st4 = sin4/4
            t47 = tpool.tile([128, NX], F16, tag="t47", name="t47")
            nc.vector.tensor_scalar_mul(out=t47, in0=Ft[4][:, 0], scalar1=4.0)  # sin4
            # s5/c5 = angle addition 1+4; s7/c7 = 3+4 (sharing c4, t47)
            tA = tpool.tile([128, NX], F16, tag="t", name="tA")
            tB = tpool.tile([128, NX], F16, tag="t", name="tB")
            nc.vector.tensor_tensor(out=tA, in0=Ft[1][:, 0], in1=Ft[4][:, 1], op=ALU.mult)
            nc.vector.tensor_tensor(out=tB, in0=Ft[1][:, 1], in1=t47, op=ALU.mult)
            nc.vector.tensor_tensor(out=Ft[5][:, 0], in0=tA, in1=tB, op=ALU.add)   # s5
            tC = tpool.tile([128, NX], F16, tag="t", name="tC")
            tD = tpool.tile([128, NX], F16, tag="t", name="tD")
            nc.vector.tensor_tensor(out=tC, in0=Ft[1][:, 1], in1=Ft[4][:, 1], op=ALU.mult)
            nc.vector.tensor_tensor(out=tD, in0=Ft[1][:, 0], in1=t47, op=ALU.mult)
            nc.vector.tensor_tensor(out=Ft[5][:, 1], in0=tC, in1=tD, op=ALU.subtract)  # c5
            nc.vector.tensor_tensor(out=Ft[6][:, 0], in0=Ft[3][:, 0], in1=Ft[3][:, 1],
                                    op=ALU.mult)                        # st6 = sin6/2
            Q3 = tpool.tile([128, NX], F16, tag="t", name="Q3")
            nc.scalar.activation(out=Q3, in_=Ft[3][:, 0], func=AF.Square)
            nc.vector.tensor_scalar(out=Ft[6][:, 1], in0=Q3, scalar1=-2.0, scalar2=1.0,
                                    op0=ALU.mult, op1=ALU.add)          # c6
            tE = tpool.tile([128, NX], F16, tag="t", name="tE")
            tF = tpool.tile([128, NX], F16, tag="t", name="tF")
            nc.vector.tensor_tensor(out=tE, in0=Ft[3][:, 0], in1=Ft[4][:, 1], op=ALU.mult)
            nc.vector.tensor_tensor(out=tF, in0=Ft[3][:, 1], in1=t47, op=ALU.mult)
            nc.vector.tensor_tensor(out=Ft[7][:, 0], in0=tE, in1=tF, op=ALU.add)   # s7
            tG = tpool.tile([128, NX], F16, tag="t", name="tG")
            tH = tpool.tile([128, NX], F16, tag="t", name="tH")
            nc.vector.tensor_tensor(out=tG, in0=Ft[3][:, 1], in1=Ft[4][:, 1], op=ALU.mult)
            nc.vector.tensor_tensor(out=tH, in0=Ft[3][:, 0], in1=t47, op=ALU.mult)
            nc.vector.tensor_tensor(out=Ft[7][:, 1], in0=tG, in1=tH, op=ALU.subtract)  # c7
            nc.vector.tensor_tensor(out=Ft[8][:, 0], in0=Ft[4][:, 0], in1=Ft[4][:, 1],
                                    op=ALU.mult)                        # st8 = sin8/8
            Q4 = tpool.tile([128, NX], F16, tag="t", name="Q4")
            nc.scalar.activation(out=Q4, in_=Ft[4][:, 0], func=AF.Square)
            nc.vector.tensor_scalar(out=Ft[8][:, 1], in0=Q4, scalar1=-32.0, scalar2=1.0,
                                    op0=ALU.mult, op1=ALU.add)          # c8
            nc.vector.tensor_tensor(out=Ft[10][:, 0], in0=Ft[5][:, 0], in1=Ft[5][:, 1],
                                    op=ALU.mult)                        # st10 = sin10/2
            Q5 = tpool.tile([128, NX], F16, tag="t", name="Q5")
            nc.scalar.activation(out=Q5, in_=Ft[5][:, 0], func=AF.Square)
            nc.vector.tensor_scalar(out=Ft[10][:, 1], in0=Q5, scalar1=-2.0, scalar2=1.0,
                                    op0=ALU.mult, op1=ALU.add)          # c10

            # ---- energy accumulation ----
            started = [False] * BPC

            def mm(b, lhsT, rhs, last=False):
                nc.tensor.matmul(e_ps[b], lhsT=lhsT, rhs=rhs,
                                 start=not started[b], stop=last)
                started[b] = True

            for k in STD_KS:
                bl = float(B_COEF[k] * LAM[k])
                for b in range(BPC):
                    for ht in range(2):
                        off = _off(b, ht)
                        sc = scpool.tile([128, 2, F], F16, tag="sc")
                        nc.vector.tensor_scalar(
                            out=sc, in0=Ft[k][:, :, off : off + F],
                            scalar1=v32[:, ht : ht + 1], scalar2=bl,
                            op0=ALU.mult, op1=ALU.mult)
                        coff = off + F
                        mm(b, sc[:, 0, :], Ft[k][:, 1, coff : coff + S])
                        mm(b, sc[:, 1, :], Ft[k][:, 0, coff : coff + S])

            n_ev = len(EVEN_KS)
            for mi, m in enumerate(EVEN_KS):
                a = m // 2
                bm = float(B_COEF[m])
                lam = LAM[a]
                last_ev = mi == n_ev - 1
                Pd = dpool.tile([128, NX], F16, tag="pd")
                nc.vector.tensor_tensor(out=Pd, in0=Ft[a][:, 0], in1=Ft[a][:, 1], op=ALU.mult)
                Qd = dpool.tile([128, NX], F16, tag="qd")
                nc.scalar.activation(out=Qd, in_=Ft[a][:, 0], func=AF.Square)
                c_t2 = float(-4.0 * bm * lam ** 3)
                c_t3 = float(2.0 * bm * lam)
                for ht in range(2):
                    ones_v = scpool.tile([128, F], F16, tag="ov")
                    nc.vector.tensor_scalar(
                        out=ones_v, in0=v32[:, ht : ht + 1].broadcast_to((128, F)),
                        scalar1=c_t3, scalar2=None, op0=ALU.mult, op1=ALU.bypass)
                    for b in range(BPC):
                        off = _off(b, ht)
                        coff = off + F
                        st2 = scpool.tile([128, F], F16, tag="st2")
                        st4 = scpool.tile([128, F], F16, tag="st4")
                        nc.vector.tensor_scalar(
                            out=st2, in0=Pd[:, off : off + F],
                            scalar1=v32[:, ht : ht + 1], scalar2=c_t2,
                            op0=ALU.mult, op1=ALU.mult)
                        nc.vector.tensor_scalar(
                            out=st4, in0=Qd[:, off : off + F],
                            scalar1=v32[:, ht : ht + 1], scalar2=c_t2,
                            op0=ALU.mult, op1=ALU.mult)
                        mm(b, st2, Qd[:, coff : coff + S])
                        mm(b, st4, Pd[:, coff : coff + S])
                        mm(b, ones_v, Pd[:, coff : coff + S], last=last_ev and ht == 1)

            # ---- softmax over s ----
            for b in range(BPC):
                negmax = stats.tile([128, 1], F32)
                nc.vector.tensor_reduce(out=negmax, in_=e_ps[b], axis=mybir.AxisListType.X,
                                        op=ALU.max, negate=True)
                p_sb = outp.tile([128, S], F32)
                ssum = stats.tile([128, 1], F32)
                nc.scalar.activation(out=p_sb, in_=e_ps[b], func=AF.Exp, bias=negmax,
                                     scale=1.0, accum_out=ssum)
                rsum = stats.tile([128, 1], F32)
                nc.vector.reciprocal(rsum, ssum)
                nc.vector.tensor_scalar_mul(p_sb, in0=p_sb, scalar1=rsum)
                nc.sync.dma_start(out=out_d[b], in_=p_sb)

    import bass_rust

    bass_rust.generate_event_semaphores(nc)
    return nc


def host_prep(query, context, W_q, W_c, v):
    """Transpose inputs; W is pre-scaled by 1/(2L) so projections emit
    phase-period units directly."""
    s = 1.0 / (2.0 * L_PERIOD)
    queryT = np.ascontiguousarray(np.transpose(query, (0, 2, 1)), dtype=np.float32)
    contextT = np.ascontiguousarray(np.transpose(context, (0, 2, 1)), dtype=np.float32)
    w_qT = np.ascontiguousarray(np.transpose(W_q) * s, dtype=np.float32)
    w_cT = np.ascontiguousarray(np.transpose(W_c) * s, dtype=np.float32)
    v2 = np.ascontiguousarray(v, dtype=np.float32).reshape(H, 1)
    return queryT, contextT, w_qT, w_cT, v2


_RUNNER_CACHE = None


def _make_runner():
    """Compile the program once; return f(concat_inputs) -> concat out."""
    import jax
    from jax.sharding import Mesh, PartitionSpec
    from jax.experimental.shard_map import shard_map
    from concourse import bass2jax

    nc = build_program()
    bass2jax.install_neuronx_cc_hook()
    partition_name = nc.partition_id_tensor.name if nc.partition_id_tensor else None
    in_names, out_names, out_avals = [], [], []
    for alloc in nc.m.functions[0].allocations:
        if not isinstance(alloc, mybir.MemoryLocationSet):
            continue
        name = alloc.memorylocations[0].name
        if alloc.kind == "ExternalInput":
            if name != partition_name:
                in_names.append(name)
        elif alloc.kind == "ExternalOutput":
            out_names.append(name)
            out_avals.append(
                jax.core.ShapedArray(tuple(alloc.tensor_shape), mybir.dt.np(alloc.dtype))
            )
    n_params = len(in_names)
    all_in_names = list(in_names) + out_names
    if partition_name is not None:
        all_in_names.append(partition_name)

    def _body(*args):
        operands = list(args)
        if partition_name is not None:
            operands.append(bass2jax.partition_id_tensor())
        return tuple(
            bass2jax._bass_exec_p.bind(
                *operands,
                out_avals=tuple(out_avals),
                in_names=tuple(all_in_names),
                out_names=tuple(out_names),
                lowering_input_output_aliases=(),
                sim_require_finite=True,
                sim_require_nnan=True,
                nc=nc,
            )
        )

    devices = jax.devices()[:NCORES]
    mesh = Mesh(np.asarray(devices), ("core",))
    n_outs = len(out_names)
    sharded = jax.jit(
        shard_map(
            _body,
            mesh=mesh,
            in_specs=(PartitionSpec("core"),) * (n_params + n_outs),
            out_specs=(PartitionSpec("core"),) * n_outs,
            check_rep=False,
        ),
        keep_unused=True,
    )
    zeros = [np.zeros((NCORES * a.shape[0], *a.shape[1:]), a.dtype) for a in out_avals]
    oi = out_names.index("out")

    def run(by_name: dict):
        args = [by_name[n] for n in in_names] + zeros
        out = sharded(*args)
        return np.asarray(out[oi])

    return run


def kernel(**inputs: np.ndarray) -> np.ndarray:
    global _RUNNER_CACHE
    queryT, contextT, w_qT, w_cT, v2 = host_prep(
        inputs["query"], inputs["context"], inputs["W_q"], inputs["W_c"], inputs["v"]
    )
    if _RUNNER_CACHE is None:
        _RUNNER_CACHE = _make_runner()
    out = _RUNNER_CACHE(
        {
            "queryT": queryT.reshape(B, D, F),
            "contextT": contextT.reshape(B, D, S),
            "w_qT": np.broadcast_to(w_qT, (NCORES, D, H)).reshape(NCORES * D, H),
            "w_cT": np.broadcast_to(w_cT, (NCORES, D, H)).reshape(NCORES * D, H),
            "v": np.broadcast_to(v2, (NCORES, H, 1)).reshape(NCORES * H, 1),
        }
    )
    return np.ascontiguousarray(out.reshape(B, F, S).astype(np.float32))


if __name__ == "__main__":
    rng = np.random.default_rng(0)
    ins = {
        "query": rng.standard_normal((B, F, D), dtype=np.float32),
        "context": rng.standard_normal((B, S, D), dtype=np.float32),
        "W_q": rng.standard_normal((H, D), dtype=np.float32) / np.sqrt(D),
        "W_c": rng.standard_normal((H, D), dtype=np.float32) / np.sqrt(D),
        "v": rng.standard_normal((H,), dtype=np.float32),
    }
    o = kernel(**ins)
    print(o.shape, o.dtype, o.sum())
